# revision 20
# baseline (speedup 1.0000x reference)
"""Contrastive-learning NCE loss on 8 trn2 NeuronCores (Bass/Tile).

Problem (hardcoded shapes): B=8, L=1024, D_in=512, D_feat=256, N=B*L=8192.
  emb_k = relu(feature_k @ W + b)                     [B, L, Df]
  positive = <e1,e2> + banded_diag_mean terms         [N]
  negative = logsumexp(e1 @ e2.T, axis=-1) - log(N)   [N]
  loss = mean(-positive + negative)

Sharding: token dim N split across 8 cores = one batch row each (L == N/8).
Each core computes its [1024, 8192] slab of the similarity matrix against the
full emb_2 (recomputed locally from full feature2). The host rotates feature2
per core (chunk-granular) so the core's own batch sits at columns 0:1023 ->
the SPMD program is core-index free.

v4 design (baseline bf16 ~160-187us -> v2 fp8 126us -> v3 119us):
  * All matmuls fp8e4m3 + DoubleRow (K=256 per MM; ~220ns per 512-col MM).
  * W (and b) are pre-scaled by sqrt(A), A = 128/ln2, so the sim PSUM holds
    A*sim: the ScalarE exp uses the free affine (scale=1/A, bias=-64), and
    offloaded groups can run a 2-op Schraudolph exp on the DVE:
      int16(max(ps + B2, 0)) bitcast bf16 == exp(sim-64) to ~1.5%,
    then tensor_reduce -> the row-sum. 4 of 32 groups go to the (otherwise
    slack) DVE, relieving the ACT roofline. Banded terms come out scaled by
    A; the host divides them back.
  * Constant logsumexp shift (-64) is exact for any shift; max sim ~120 keeps
    exp in fp32 range. exp is computed in-place on PSUM with the fused row
    accumulator; host adds 64 + log(S).
  * Inputs are staged host-side into partition-major chunk-contiguous
    layouts so every DMA moves 4KB per partition-line (128 descriptors vs
    512 thin ones) -- the first projection starts ~5us earlier.
  * One software pipeline paced by ACT: the 2x[128,2048] PSUM ring is shared
    by sim groups and projection tiles, with projection d-tiles interleaved
    INSIDE the sim m-loop. Prologue projection consumers run on the ACT
    (Relu lives in the same table set as Exp), leaving the DVE free.
  * Banded-term DVE work is queued as single-op closures drained on a
    per-block quota at the projection-consumer points; the tiny PE
    reduce-MMs slot into block 3.
  * A PE warmup burst during the DMA head defeats the HAM cold-clock.
"""

import numpy as np
import ml_dtypes
from collections import deque
from contextlib import ExitStack

import concourse.bass as bass
import concourse.tile as tile
from concourse import bacc, mybir
from concourse import bass_isa
from concourse import bass_utils

dt = mybir.dt
AF = mybir.ActivationFunctionType
ALU = mybir.AluOpType
PM = mybir.MatmulPerfMode

N_CORES = 8
B, L, DIN, DF = 8, 1024, 512, 256
N = B * L
KO = DIN // 128     # 4 k-tiles of the projection contraction
NDT = DF // 128     # 2 d-tiles of the embedding dim
PAD = 4             # box-filter padding (max supported positive_range)
LP = L + 2 * PAD
CW = 2048           # sim-phase column group (one PSUM tile / one ACT)
NCG = N // CW       # 4 sim blocks
SHIFT = 64.0        # constant logsumexp shift

EXPA = 128.0 / np.log(2.0)          # 184.665 = Schraudolph slope
SQA = float(np.sqrt(EXPA))          # weight pre-scale
SIGMA = 8.13                        # Schraudolph bias tuning (HW-calibrated)
B2 = 16256.0 - SIGMA - EXPA * SHIFT
OFFLOAD = set()   # (block, m) exp groups to run on the DVE (Schraudolph);
                  # empty: the pipeline is PE-bound under 8-core power
                  # throttling, so offloading ACT work buys nothing and the
                  # slower DVE slot turnaround costs PE pacing

_module_cache = {}


def _box_terms(w: int):
    """Decompose window width w (odd, <= 2*PAD+1) into power-of-2 segments."""
    terms, off = [], 0
    for p in (8, 4, 2, 1):
        if w >= p:
            terms.append((p, off))
            off += p
            w -= p
    assert w == 0
    return terms


def _build(r_self: int, r_tgt: int):
    nc = bacc.Bacc("TRN2", target_bir_lowering=False, debug=False, num_devices=N_CORES)

    # partition-major staged inputs (see _make_in_maps)
    f1t = nc.dram_tensor("f1t", [128, KO * L], dt.float8e4, kind="ExternalInput").ap()
    f2t = nc.dram_tensor("f2t", [128, B * KO * L], dt.float8e4, kind="ExternalInput").ap()
    w_in = nc.dram_tensor("w_in", [128, KO * DF], dt.float8e4, kind="ExternalInput").ap()
    b_in = nc.dram_tensor("b_in", [DF], dt.float32, kind="ExternalInput").ap()

    pos_main = nc.dram_tensor("pos_main", [L], dt.float32, kind="ExternalOutput").ap()
    pos_self = nc.dram_tensor("pos_self", [L], dt.float32, kind="ExternalOutput").ap()
    pos_tgt = nc.dram_tensor("pos_tgt", [L], dt.float32, kind="ExternalOutput").ap()
    s_out_a = nc.dram_tensor("s_out_a", [128, 8 * 3], dt.float32, kind="ExternalOutput").ap()
    s_out_b = nc.dram_tensor("s_out_b", [128, 8], dt.float32, kind="ExternalOutput").ap()

    with tile.TileContext(nc) as tc, ExitStack() as ctx:
        const = ctx.enter_context(tc.tile_pool(name="const", bufs=1))
        stage = ctx.enter_context(tc.tile_pool(name="stage", bufs=3))
        emb = ctx.enter_context(tc.tile_pool(name="emb", bufs=1))
        band = ctx.enter_context(tc.tile_pool(name="band", bufs=1))
        prodp = ctx.enter_context(tc.tile_pool(name="prodp", bufs=8))
        rows = ctx.enter_context(tc.tile_pool(name="rows", bufs=1))
        mmp = ctx.enter_context(tc.tile_pool(name="mmp", bufs=2, space="PSUM"))

        # ---- constants -------------------------------------------------
        wt = const.tile([128, KO, DF], dt.float8e4)    # W[ko*128+p, d] at [p, ko, d]
        nc.sync.dma_start(out=wt[:], in_=w_in[:].rearrange("p (ko d) -> p ko d", ko=KO))
        b_col = const.tile([128, NDT], dt.float32)     # bias per (d%128, dtile)
        nc.sync.dma_start(out=b_col[:], in_=b_in[:].rearrange("(d p) -> p d", p=128))
        ones_f = const.tile([128, 1], dt.float32)
        nc.vector.memset(ones_f[:], 1.0)
        ones = const.tile([128, 1], dt.bfloat16)
        nc.vector.tensor_copy(ones[:], ones_f[:])
        neg_shift = const.tile([128, 1], dt.float32)
        nc.vector.memset(neg_shift[:], -SHIFT)
        warm = const.tile([128, 1], dt.float32)
        # dummy exp: the ACT exp-table load happens during the DMA head
        nc.scalar.activation(warm[:], ones_f[:], AF.Exp, bias=neg_shift[:], scale=1.0)

        # PE warmup: junk matmuls during the DMA head keep the HAM activity
        # monitor busy so real MMs run at 2.4GHz from the start
        wst = const.tile([128, 128], dt.bfloat16)
        wmv = const.tile([128, 512], dt.bfloat16)
        nc.vector.memset(wst[:], 0.25)
        nc.vector.memset(wmv[:], 0.25)

        # ---- embeddings (all scaled by sqrt(A)) ------------------------
        e1q = emb.tile([128, NDT, L], dt.float8e4, name="e1q", tag="e1q")
        e2q = emb.tile([128, NDT, N], dt.float8e4, name="e2q", tag="e2q")
        e1b = emb.tile([128, NDT, LP], dt.bfloat16, name="e1b", tag="e1b")
        e2b = emb.tile([128, NDT, LP], dt.bfloat16, name="e2b", tag="e2b")
        nc.vector.memzero(e1b[:])
        nc.vector.memzero(e2b[:])

        stot = const.tile([128, 8 * NCG], dt.float32)   # [p, m*NCG + bk]
        t16 = const.tile([128, CW], dt.int16)           # Schraudolph scratch

        # ---- projection pieces -----------------------------------------
        def pj_mm(src_view, d, tag):
            """PE half of a projection d-tile (chunk already staged)."""
            ps = mmp.tile([128, L], dt.float32, tag="mm", name=f"pj_{tag}_{d}")
            for kk in range(KO // 2):
                for h in range(L // 512):
                    nc.tensor.matmul(
                        ps[:, h * 512:(h + 1) * 512],
                        wt[:, 2 * kk:2 * kk + 2, d * 128:(d + 1) * 128],
                        src_view[:, 2 * kk:2 * kk + 2, h * 512:(h + 1) * 512],
                        start=(kk == 0), stop=(kk == KO // 2 - 1),
                        perf_mode=PM.DoubleRow)
            return ps

        def stage_chunk(src_ap, col0, tag):
            fst = stage.tile([128, KO, L], dt.float8e4, tag="fstage", name=f"fst_{tag}")
            nc.sync.dma_start(
                out=fst[:],
                in_=src_ap[:, col0 * KO:(col0 + L) * KO]
                    .rearrange("p (ko n) -> p ko n", ko=KO))
            return fst

        def pj_cons_dve(ps, q_dst, d, q_col0):
            nc.vector.tensor_scalar(
                q_dst[:, d, q_col0:q_col0 + L], ps[:],
                b_col[:, d:d + 1], 0.0, ALU.add, ALU.max)

        def pj_cons_act(ps, q_dst, d, q_col0):
            nc.scalar.activation(q_dst[:, d, q_col0:q_col0 + L], ps[:],
                                 AF.Relu, bias=b_col[:, d:d + 1], scale=1.0)

        # ---- sim group --------------------------------------------------
        def sim_group(bk, m):
            ps = mmp.tile([128, CW], dt.float32, tag="mm", name=f"sim_{bk}_{m}")
            for q in range(CW // 512):
                nc.tensor.matmul(
                    ps[:, q * 512:(q + 1) * 512],
                    e1q[:, :, m * 128:(m + 1) * 128],
                    e2q[:, :, bk * CW + q * 512: bk * CW + (q + 1) * 512],
                    start=True, stop=True, perf_mode=PM.DoubleRow)
            col = stot[:, m * NCG + bk: m * NCG + bk + 1]
            if (bk, m) in OFFLOAD:
                # DVE Schraudolph: exp(sim-64) ~= bf16_bits(A*sim + B2)
                nc.vector.tensor_scalar(t16[:], ps[:], B2, 0.0, ALU.add, ALU.max)
                nc.vector.tensor_reduce(col, t16[:].bitcast(dt.bfloat16),
                                        mybir.AxisListType.X, ALU.add)
            else:
                nc.scalar.activation(ps[:], ps[:], AF.Exp,
                                     bias=neg_shift[:], scale=1.0 / EXPA,
                                     accum_out=col)

        # ---- banded-term closures (drained on per-block quotas) ---------
        boxes = {}
        prods = {"main": [], "self": [], "tgt": []}
        dq = deque()

        def mk_copy(dst, src_q, d):
            def f():
                nc.vector.tensor_copy(dst[:, d, PAD:PAD + L], src_q[:, d, 0:L])
            return f

        def queue_boxsum(key, src, d, r):
            """Queue the box-filter as single-op closures; stores result view."""
            wdt = 2 * r + 1
            state = {1: src[:, d, :]}

            def mk_shift(p):
                def f():
                    sp = band.tile([128, LP], dt.bfloat16, name=f"s{p}_{key}_{d}",
                                   tag=f"s{p}")
                    h = p // 2
                    nv = LP - p + 1
                    nc.vector.tensor_tensor(sp[:, :nv], state[h][:, :nv],
                                            state[h][:, h:h + nv], ALU.add)
                    state[p] = sp
                return f
            for p in (2, 4, 8):
                if wdt >= p:
                    dq.append(mk_shift(p))

            def mk_fin():
                def f():
                    terms = _box_terms(wdt)
                    t0 = PAD - r
                    if len(terms) == 1:
                        p0, o0 = terms[0]
                        boxes[(key, d)] = state[p0][:, t0 + o0: t0 + o0 + L]
                        return
                    acc = band.tile([128, L], dt.bfloat16, name=f"box_{key}_{d}",
                                    tag="box", bufs=6)
                    p0, o0 = terms[0]
                    p1, o1 = terms[1]
                    nc.vector.tensor_tensor(acc[:], state[p0][:, t0 + o0: t0 + o0 + L],
                                            state[p1][:, t0 + o1: t0 + o1 + L], ALU.add)
                    for p, o in terms[2:]:
                        nc.vector.tensor_tensor(acc[:], acc[:],
                                                state[p][:, t0 + o: t0 + o + L], ALU.add)
                    boxes[(key, d)] = acc[:]
                return f
            dq.append(mk_fin())

        def mk_prod(key, gi, a_fn, b_fn):
            def f():
                prod = prodp.tile([128, L], dt.bfloat16, tag=f"prod_{key}_{gi}", bufs=1)
                nc.vector.tensor_tensor(prod[:], a_fn(), b_fn(), ALU.mult)
                prods[key].append(prod)
            return f

        e1v = [e1b[:, d, PAD:PAD + L] for d in range(NDT)]
        e2v = [e2b[:, d, PAD:PAD + L] for d in range(NDT)]

        for d in range(NDT):
            dq.append(mk_copy(e1b, e1q, d))
        for d in range(NDT):
            dq.append(mk_copy(e2b, e2q, d))
        if r_self:
            for d in range(NDT):
                queue_boxsum("bx1", e1b, d, r_self)
            for d in range(NDT):
                queue_boxsum("bx2", e2b, d, r_self)
        if r_tgt and r_tgt != r_self:
            for d in range(NDT):
                queue_boxsum("bxt", e2b, d, r_tgt)
        tkey = "bxt" if (r_tgt and r_tgt != r_self) else "bx2"
        for d in range(NDT):
            dq.append(mk_prod("main", d, lambda d=d: e1v[d], lambda d=d: e2v[d]))
        if r_self:
            for d in range(NDT):
                dq.append(mk_prod("self", d, lambda d=d: e1v[d],
                                  lambda d=d: boxes[("bx1", d)]))
            for d in range(NDT):
                dq.append(mk_prod("self", NDT + d, lambda d=d: e2v[d],
                                  lambda d=d: boxes[("bx2", d)]))
        if r_tgt:
            for d in range(NDT):
                dq.append(mk_prod("tgt", d, lambda d=d: e1v[d],
                                  lambda d=d: boxes[(tkey, d)]))

        def drain(k):
            for _ in range(k):
                if dq:
                    dq.popleft()()

        # ---- pos reduce-MMs + row evacuation (block 3) -------------------
        rps = {}

        def mk_reduce_mm(key):
            # matmul out must fit one PSUM bank (512 fp32) -> two 512 halves
            def f():
                rp = mmp.tile([1, L], dt.float32, tag="mm", name=f"rp_{key}")
                pr = prods[key]
                for h in range(L // 512):
                    for gi, prod in enumerate(pr):
                        nc.tensor.matmul(rp[:, h * 512:(h + 1) * 512], ones[:],
                                         prod[:, h * 512:(h + 1) * 512],
                                         start=(gi == 0), stop=(gi == len(pr) - 1))
                rps[key] = rp
            return f

        def mk_row(key, out_dram):
            def f():
                row = rows.tile([1, L], dt.float32, tag=f"row_{key}")
                nc.vector.tensor_copy(row[:], rps[key][:])
                nc.sync.dma_start(out=out_dram[:].rearrange("(one n) -> one n", one=1),
                                  in_=row[:])
            return f

        def zero_out(out_dram, tag):
            zr = rows.tile([1, L], dt.float32, tag=f"zr_{tag}")
            nc.vector.memset(zr[:], 0.0)
            nc.sync.dma_start(out=out_dram[:].rearrange("(one n) -> one n", one=1),
                              in_=zr[:])

        red_list = [("main", pos_main)]
        if r_self:
            red_list.append(("self", pos_self))
        else:
            zero_out(pos_self, "self")
        if r_tgt:
            red_list.append(("tgt", pos_tgt))
        else:
            zero_out(pos_tgt, "tgt")

        # ---- schedule ---------------------------------------------------
        for w in range(2):
            wps = mmp.tile([128, 512], dt.float32, tag="mm", name=f"wps_{w}")
            for _ in range(4):
                nc.tensor.matmul(wps[:], wst[:], wmv[:], start=True, stop=True)

        # prologue: f1 + e2 chunks 0,1; relu consumers on ACT (same table set)
        fst_f1 = stage_chunk(f1t, 0, "f1")
        fst_c0 = stage_chunk(f2t, 0, "c0")
        fst_c1 = stage_chunk(f2t, L, "c1")
        for d in range(NDT):
            pj_cons_act(pj_mm(fst_f1, d, "f1"), e1q, d, 0)
        for d in range(NDT):
            pj_cons_act(pj_mm(fst_c0, d, "c0"), e2q, d, 0)
        for d in range(NDT):
            pj_cons_act(pj_mm(fst_c1, d, "c1"), e2q, d, L)

        # per-block DVE drain quotas (ops per projection-consumer point)
        quota = {0: 1, 1: 4, 2: 2}

        for bk in range(NCG):
            mm_slots = {}
            dv_slots = {}
            if bk < 3:
                c0, c1 = 2 * bk + 2, 2 * bk + 3
                # prefetch both chunks' staging DMAs at block start
                fsts = {c0: stage_chunk(f2t, c0 * L, f"c{c0}"),
                        c1: stage_chunk(f2t, c1 * L, f"c{c1}")}
                for idx, (cc, d) in enumerate([(c0, 0), (c0, 1), (c1, 0), (c1, 1)]):
                    m_at = (2, 4, 6, 7)[idx]

                    def mk(cc=cc, d=d, bk=bk):
                        def f():
                            ps = pj_mm(fsts[cc], d, f"c{cc}")
                            pj_cons_dve(ps, e2q, d, cc * L)
                            drain(quota[bk])
                        return f
                    mm_slots[m_at] = mk()
            else:
                for idx, (key, out_dram) in enumerate(red_list):
                    mm_slots[2 * idx + 1] = mk_reduce_mm(key)
                    dv_slots[2 * idx + 3] = mk_row(key, out_dram)
            for m in range(8):
                sim_group(bk, m)
                if m in mm_slots:
                    mm_slots[m]()
                if m in dv_slots:
                    dv_slots[m]()
            if bk == 2:
                drain(len(dq))   # leftover banded work
                # blocks 0-2 stot columns ship now; the thin-line DMA cost
                # (128 descriptors) hides under block 3
                nc.sync.dma_start(
                    out=s_out_a[:],
                    in_=stot[:].rearrange("p (m c) -> p m c", c=NCG)[:, :, 0:3])

        nc.sync.dma_start(
            out=s_out_b[:],
            in_=stot[:].rearrange("p (m c) -> p m c", c=NCG)[:, :, 3])

    nc.compile()
    return nc


def kernel(feature1, feature2, W, b, positive_range_self, positive_range_tgt):
    r_self = int(np.asarray(positive_range_self))
    r_tgt = int(np.asarray(positive_range_tgt))
    assert 0 <= r_self <= PAD and 0 <= r_tgt <= PAD

    key = (r_self, r_tgt)
    if key not in _module_cache:
        _module_cache[key] = _build(r_self, r_tgt)
    nc = _module_cache[key]

    in_maps = _make_in_maps(feature1, feature2, W, b)
    res = bass_utils.run_bass_kernel_spmd(nc, in_maps, list(range(N_CORES)))

    # ---- host combine (fp64) ---------------------------------------------
    j = np.arange(L)
    loss_terms = []
    for i in range(N_CORES):
        r = res.results[i]
        Sa = r["s_out_a"].astype(np.float64).reshape(128, 8, 3).sum(axis=2)
        S = (Sa + r["s_out_b"].astype(np.float64)).T.reshape(L)
        t = SHIFT + np.log(S) - np.log(float(N))             # negative_j
        t -= r["pos_main"].astype(np.float64) / EXPA
        if r_self > 0:
            cnt = np.minimum(L - 1, j + r_self) - np.maximum(0, j - r_self) + 1.0
            t -= r["pos_self"].astype(np.float64) / EXPA / cnt
        if r_tgt > 0:
            cnt = np.minimum(L - 1, j + r_tgt) - np.maximum(0, j - r_tgt) + 1.0
            t -= r["pos_tgt"].astype(np.float64) / EXPA / cnt
        loss_terms.append(t)
    loss = np.mean(np.concatenate(loss_terms))
    return np.float32(loss)


def _make_in_maps(feature1, feature2, W, b):
    fp8 = ml_dtypes.float8_e4m3
    f1 = np.asarray(feature1, dtype=np.float32)
    f2 = np.asarray(feature2, dtype=np.float32)
    Wr = np.asarray(W, dtype=np.float32) * SQA
    bv = np.ascontiguousarray(np.asarray(b, dtype=np.float32) * SQA)

    # W: [DIN, DF] -> [128, KO*DF] partition-major (w[p, ko*DF+d] = W[ko*128+p, d])
    w_pm = np.ascontiguousarray(
        Wr.reshape(KO, 128, DF).transpose(1, 0, 2).reshape(128, KO * DF).astype(fp8))

    # f2: [B, L, DIN] -> [128, c, KO, L] chunk-contiguous partition-major
    f2T = f2.reshape(N, DIN).T.astype(fp8)                   # [DIN, N]
    f2_4d = (f2T.reshape(KO, 128, B, L).transpose(1, 2, 0, 3))   # [p, c, ko, n]
    in_maps = []
    for i in range(N_CORES):
        f1T = f1[i].T.astype(fp8)                            # [DIN, L]
        f1_pm = np.ascontiguousarray(
            f1T.reshape(KO, 128, L).transpose(1, 0, 2).reshape(128, KO * L))
        f2_rot = np.ascontiguousarray(
            np.roll(f2_4d, -i, axis=1).reshape(128, B * KO * L))
        in_maps.append({"f1t": f1_pm, "f2t": f2_rot, "w_in": w_pm, "b_in": bv})
    return in_maps


# revision 26
# speedup vs baseline: 1.0055x; 1.0055x over previous
"""Contrastive-learning NCE loss on 8 trn2 NeuronCores (Bass/Tile).

Problem (hardcoded shapes): B=8, L=1024, D_in=512, D_feat=256, N=B*L=8192.
  emb_k = relu(feature_k @ W + b)                     [B, L, Df]
  positive = <e1,e2> + banded_diag_mean terms         [N]
  negative = logsumexp(e1 @ e2.T, axis=-1) - log(N)   [N]
  loss = mean(-positive + negative)

Sharding: token dim N split across 8 cores = one batch row each (L == N/8).
Each core computes its [1024, 8192] slab of the similarity matrix against the
full emb_2 (recomputed locally from full feature2). The host rotates feature2
per core (chunk-granular) so the core's own batch sits at columns 0:1023 ->
the SPMD program is core-index free.

v4 design (baseline bf16 ~160-187us -> v2 fp8 126us -> v3 119us):
  * All matmuls fp8e4m3 + DoubleRow (K=256 per MM; ~220ns per 512-col MM).
  * W (and b) are pre-scaled by sqrt(A), A = 128/ln2, so the sim PSUM holds
    A*sim: the ScalarE exp uses the free affine (scale=1/A, bias=-64), and
    offloaded groups can run a 2-op Schraudolph exp on the DVE:
      int16(max(ps + B2, 0)) bitcast bf16 == exp(sim-64) to ~1.5%,
    then tensor_reduce -> the row-sum. 4 of 32 groups go to the (otherwise
    slack) DVE, relieving the ACT roofline. Banded terms come out scaled by
    A; the host divides them back.
  * Constant logsumexp shift (-64) is exact for any shift; max sim ~120 keeps
    exp in fp32 range. exp is computed in-place on PSUM with the fused row
    accumulator; host adds 64 + log(S).
  * Inputs are staged host-side into partition-major chunk-contiguous
    layouts so every DMA moves 4KB per partition-line (128 descriptors vs
    512 thin ones) -- the first projection starts ~5us earlier.
  * One software pipeline paced by ACT: the 2x[128,2048] PSUM ring is shared
    by sim groups and projection tiles, with projection d-tiles interleaved
    INSIDE the sim m-loop. Prologue projection consumers run on the ACT
    (Relu lives in the same table set as Exp), leaving the DVE free.
  * Banded-term DVE work is queued as single-op closures drained on a
    per-block quota at the projection-consumer points; the tiny PE
    reduce-MMs slot into block 3.
  * A PE warmup burst during the DMA head defeats the HAM cold-clock.
"""

import numpy as np
import ml_dtypes
from collections import deque
from contextlib import ExitStack

import concourse.bass as bass
import concourse.tile as tile
from concourse import bacc, mybir
from concourse import bass_utils

dt = mybir.dt
AF = mybir.ActivationFunctionType
ALU = mybir.AluOpType
PM = mybir.MatmulPerfMode

N_CORES = 8
B, L, DIN, DF = 8, 1024, 512, 256
N = B * L
KO = DIN // 128     # 4 k-tiles of the projection contraction
NDT = DF // 128     # 2 d-tiles of the embedding dim
PAD = 4             # box-filter padding (max supported positive_range)
LP = L + 2 * PAD
CW = 2048           # sim-phase column group (one PSUM tile / one ACT)
NCG = N // CW       # 4 sim blocks
SHIFT = 64.0        # constant logsumexp shift

EXPA = 128.0 / np.log(2.0)          # 184.665 = Schraudolph slope
SQA = float(np.sqrt(EXPA))          # weight pre-scale
SIGMA = 8.13                        # Schraudolph bias tuning (HW-calibrated)
B2 = 16256.0 - SIGMA - EXPA * SHIFT
OFFLOAD = {(0, 0), (2, 0), (3, 0), (3, 4)}   # (block, m) exp groups on the DVE

_module_cache = {}


def _box_terms(w: int):
    """Decompose window width w (odd, <= 2*PAD+1) into power-of-2 segments."""
    terms, off = [], 0
    for p in (8, 4, 2, 1):
        if w >= p:
            terms.append((p, off))
            off += p
            w -= p
    assert w == 0
    return terms


def _build(r_self: int, r_tgt: int):
    nc = bacc.Bacc("TRN2", target_bir_lowering=False, debug=False, num_devices=N_CORES)

    # partition-major staged inputs (see _make_in_maps)
    f1t = nc.dram_tensor("f1t", [128, KO * L], dt.float8e4, kind="ExternalInput").ap()
    f2t = nc.dram_tensor("f2t", [128, B * KO * L], dt.float8e4, kind="ExternalInput").ap()
    w_in = nc.dram_tensor("w_in", [128, KO * DF], dt.float8e4, kind="ExternalInput").ap()
    b_in = nc.dram_tensor("b_in", [DF], dt.float32, kind="ExternalInput").ap()

    pos_main = nc.dram_tensor("pos_main", [L], dt.float32, kind="ExternalOutput").ap()
    pos_self = nc.dram_tensor("pos_self", [L], dt.float32, kind="ExternalOutput").ap()
    pos_tgt = nc.dram_tensor("pos_tgt", [L], dt.float32, kind="ExternalOutput").ap()
    s_out_a = nc.dram_tensor("s_out_a", [128, 8 * 3], dt.float32, kind="ExternalOutput").ap()
    s_out_b = nc.dram_tensor("s_out_b", [128, 8], dt.float32, kind="ExternalOutput").ap()

    with tile.TileContext(nc) as tc, ExitStack() as ctx:
        const = ctx.enter_context(tc.tile_pool(name="const", bufs=1))
        stage = ctx.enter_context(tc.tile_pool(name="stage", bufs=3))
        emb = ctx.enter_context(tc.tile_pool(name="emb", bufs=1))
        band = ctx.enter_context(tc.tile_pool(name="band", bufs=1))
        prodp = ctx.enter_context(tc.tile_pool(name="prodp", bufs=8))
        rows = ctx.enter_context(tc.tile_pool(name="rows", bufs=1))
        mmp = ctx.enter_context(tc.tile_pool(name="mmp", bufs=2, space="PSUM"))

        # ---- constants -------------------------------------------------
        wt = const.tile([128, KO, DF], dt.float8e4)    # W[ko*128+p, d] at [p, ko, d]
        nc.sync.dma_start(out=wt[:], in_=w_in[:].rearrange("p (ko d) -> p ko d", ko=KO))
        b_col = const.tile([128, NDT], dt.float32)     # bias per (d%128, dtile)
        nc.sync.dma_start(out=b_col[:], in_=b_in[:].rearrange("(d p) -> p d", p=128))
        ones_f = const.tile([128, 1], dt.float32)
        nc.vector.memset(ones_f[:], 1.0)
        ones = const.tile([128, 1], dt.bfloat16)
        nc.vector.tensor_copy(ones[:], ones_f[:])
        neg_shift = const.tile([128, 1], dt.float32)
        nc.vector.memset(neg_shift[:], -SHIFT)
        warm = const.tile([128, 1], dt.float32)
        # dummy exp: the ACT exp-table load happens during the DMA head
        nc.scalar.activation(warm[:], ones_f[:], AF.Exp, bias=neg_shift[:], scale=1.0)

        # PE warmup: junk matmuls during the DMA head keep the HAM activity
        # monitor busy so real MMs run at 2.4GHz from the start
        wst = const.tile([128, 128], dt.bfloat16)
        wmv = const.tile([128, 512], dt.bfloat16)
        nc.vector.memset(wst[:], 0.25)
        nc.vector.memset(wmv[:], 0.25)

        # ---- embeddings (all scaled by sqrt(A)) ------------------------
        e1q = emb.tile([128, NDT, L], dt.float8e4, name="e1q", tag="e1q")
        e2q = emb.tile([128, NDT, N], dt.float8e4, name="e2q", tag="e2q")
        e1b = emb.tile([128, NDT, LP], dt.bfloat16, name="e1b", tag="e1b")
        e2b = emb.tile([128, NDT, LP], dt.bfloat16, name="e2b", tag="e2b")
        nc.vector.memzero(e1b[:])
        nc.vector.memzero(e2b[:])

        stot = const.tile([128, 8 * NCG], dt.float32)   # [p, m*NCG + bk]
        t16 = const.tile([128, CW], dt.int16)           # Schraudolph scratch

        # ---- projection pieces -----------------------------------------
        def pj_mm(src_view, d, tag):
            """PE half of a projection d-tile (chunk already staged)."""
            ps = mmp.tile([128, L], dt.float32, tag="mm", name=f"pj_{tag}_{d}")
            for kk in range(KO // 2):
                for h in range(L // 512):
                    nc.tensor.matmul(
                        ps[:, h * 512:(h + 1) * 512],
                        wt[:, 2 * kk:2 * kk + 2, d * 128:(d + 1) * 128],
                        src_view[:, 2 * kk:2 * kk + 2, h * 512:(h + 1) * 512],
                        start=(kk == 0), stop=(kk == KO // 2 - 1),
                        perf_mode=PM.DoubleRow)
            return ps

        def stage_chunk(src_ap, col0, tag):
            fst = stage.tile([128, KO, L], dt.float8e4, tag="fstage", name=f"fst_{tag}")
            nc.sync.dma_start(
                out=fst[:],
                in_=src_ap[:, col0 * KO:(col0 + L) * KO]
                    .rearrange("p (ko n) -> p ko n", ko=KO))
            return fst

        def pj_cons_dve(ps, q_dst, d, q_col0):
            nc.vector.tensor_scalar(
                q_dst[:, d, q_col0:q_col0 + L], ps[:],
                b_col[:, d:d + 1], 0.0, ALU.add, ALU.max)

        def pj_cons_act(ps, q_dst, d, q_col0):
            nc.scalar.activation(q_dst[:, d, q_col0:q_col0 + L], ps[:],
                                 AF.Relu, bias=b_col[:, d:d + 1], scale=1.0)

        # ---- sim group --------------------------------------------------
        def sim_group(bk, m):
            ps = mmp.tile([128, CW], dt.float32, tag="mm", name=f"sim_{bk}_{m}")
            for q in range(CW // 512):
                nc.tensor.matmul(
                    ps[:, q * 512:(q + 1) * 512],
                    e1q[:, :, m * 128:(m + 1) * 128],
                    e2q[:, :, bk * CW + q * 512: bk * CW + (q + 1) * 512],
                    start=True, stop=True, perf_mode=PM.DoubleRow)
            col = stot[:, m * NCG + bk: m * NCG + bk + 1]
            if (bk, m) in OFFLOAD:
                # DVE Schraudolph: exp(sim-64) ~= bf16_bits(A*sim + B2)
                nc.vector.tensor_scalar(t16[:], ps[:], B2, 0.0, ALU.add, ALU.max)
                nc.vector.tensor_reduce(col, t16[:].bitcast(dt.bfloat16),
                                        mybir.AxisListType.X, ALU.add)
            else:
                nc.scalar.activation(ps[:], ps[:], AF.Exp,
                                     bias=neg_shift[:], scale=1.0 / EXPA,
                                     accum_out=col)

        # ---- banded-term closures (drained on per-block quotas) ---------
        boxes = {}
        prods = {"main": [], "self": [], "tgt": []}
        dq = deque()

        def mk_copy(dst, src_q, d):
            def f():
                nc.vector.tensor_copy(dst[:, d, PAD:PAD + L], src_q[:, d, 0:L])
            return f

        def queue_boxsum(key, src, d, r):
            """Queue the box-filter as single-op closures; stores result view."""
            wdt = 2 * r + 1
            state = {1: src[:, d, :]}

            def mk_shift(p):
                def f():
                    sp = band.tile([128, LP], dt.bfloat16, name=f"s{p}_{key}_{d}",
                                   tag=f"s{p}")
                    h = p // 2
                    nv = LP - p + 1
                    nc.vector.tensor_tensor(sp[:, :nv], state[h][:, :nv],
                                            state[h][:, h:h + nv], ALU.add)
                    state[p] = sp
                return f
            for p in (2, 4, 8):
                if wdt >= p:
                    dq.append(mk_shift(p))

            def mk_fin():
                def f():
                    terms = _box_terms(wdt)
                    t0 = PAD - r
                    if len(terms) == 1:
                        p0, o0 = terms[0]
                        boxes[(key, d)] = state[p0][:, t0 + o0: t0 + o0 + L]
                        return
                    acc = band.tile([128, L], dt.bfloat16, name=f"box_{key}_{d}",
                                    tag="box", bufs=6)
                    p0, o0 = terms[0]
                    p1, o1 = terms[1]
                    nc.vector.tensor_tensor(acc[:], state[p0][:, t0 + o0: t0 + o0 + L],
                                            state[p1][:, t0 + o1: t0 + o1 + L], ALU.add)
                    for p, o in terms[2:]:
                        nc.vector.tensor_tensor(acc[:], acc[:],
                                                state[p][:, t0 + o: t0 + o + L], ALU.add)
                    boxes[(key, d)] = acc[:]
                return f
            dq.append(mk_fin())

        def mk_prod(key, gi, a_fn, b_fn):
            def f():
                prod = prodp.tile([128, L], dt.bfloat16, tag=f"prod_{key}_{gi}", bufs=1)
                nc.vector.tensor_tensor(prod[:], a_fn(), b_fn(), ALU.mult)
                prods[key].append(prod)
            return f

        e1v = [e1b[:, d, PAD:PAD + L] for d in range(NDT)]
        e2v = [e2b[:, d, PAD:PAD + L] for d in range(NDT)]

        for d in range(NDT):
            dq.append(mk_copy(e1b, e1q, d))
        for d in range(NDT):
            dq.append(mk_copy(e2b, e2q, d))
        if r_self:
            for d in range(NDT):
                queue_boxsum("bx1", e1b, d, r_self)
            for d in range(NDT):
                queue_boxsum("bx2", e2b, d, r_self)
        if r_tgt and r_tgt != r_self:
            for d in range(NDT):
                queue_boxsum("bxt", e2b, d, r_tgt)
        tkey = "bxt" if (r_tgt and r_tgt != r_self) else "bx2"
        for d in range(NDT):
            dq.append(mk_prod("main", d, lambda d=d: e1v[d], lambda d=d: e2v[d]))
        if r_self:
            for d in range(NDT):
                dq.append(mk_prod("self", d, lambda d=d: e1v[d],
                                  lambda d=d: boxes[("bx1", d)]))
            for d in range(NDT):
                dq.append(mk_prod("self", NDT + d, lambda d=d: e2v[d],
                                  lambda d=d: boxes[("bx2", d)]))
        if r_tgt:
            for d in range(NDT):
                dq.append(mk_prod("tgt", d, lambda d=d: e1v[d],
                                  lambda d=d: boxes[(tkey, d)]))

        def drain(k):
            for _ in range(k):
                if dq:
                    dq.popleft()()

        # ---- pos reduce-MMs + row evacuation (block 3) -------------------
        rps = {}
        combined = {}

        def mk_combine(key):
            """Elementwise-sum the group's prods so the PE reduce is 2 MMs."""
            def f():
                pr = prods[key]
                if len(pr) == 1:
                    combined[key] = pr[0]
                    return
                acc = prodp.tile([128, L], dt.bfloat16, tag=f"comb_{key}", bufs=1)
                nc.vector.tensor_tensor(acc[:], pr[0][:], pr[1][:], ALU.add)
                for p in pr[2:]:
                    nc.vector.tensor_tensor(acc[:], acc[:], p[:], ALU.add)
                combined[key] = acc
            return f

        def mk_reduce_mm(key):
            # matmul out must fit one PSUM bank (512 fp32) -> two 512 halves
            def f():
                rp = mmp.tile([1, L], dt.float32, tag="mm", name=f"rp_{key}")
                cp = combined[key]
                for h in range(L // 512):
                    nc.tensor.matmul(rp[:, h * 512:(h + 1) * 512], ones[:],
                                     cp[:, h * 512:(h + 1) * 512],
                                     start=True, stop=True)
                rps[key] = rp
            return f

        def mk_row(key, out_dram):
            def f():
                row = rows.tile([1, L], dt.float32, tag=f"row_{key}")
                nc.vector.tensor_copy(row[:], rps[key][:])
                nc.sync.dma_start(out=out_dram[:].rearrange("(one n) -> one n", one=1),
                                  in_=row[:])
            return f

        def zero_out(out_dram, tag):
            zr = rows.tile([1, L], dt.float32, tag=f"zr_{tag}")
            nc.vector.memset(zr[:], 0.0)
            nc.sync.dma_start(out=out_dram[:].rearrange("(one n) -> one n", one=1),
                              in_=zr[:])

        red_list = [("main", pos_main)]
        if r_self:
            red_list.append(("self", pos_self))
        else:
            zero_out(pos_self, "self")
        if r_tgt:
            red_list.append(("tgt", pos_tgt))
        else:
            zero_out(pos_tgt, "tgt")
        for key, _ in red_list:
            dq.append(mk_combine(key))

        # ---- schedule ---------------------------------------------------
        for w in range(2):
            wps = mmp.tile([128, 512], dt.float32, tag="mm", name=f"wps_{w}")
            for _ in range(4):
                nc.tensor.matmul(wps[:], wst[:], wmv[:], start=True, stop=True)

        # prologue: f1 + e2 chunks 0,1; relu consumers on ACT (same table set)
        fst_f1 = stage_chunk(f1t, 0, "f1")
        fst_c0 = stage_chunk(f2t, 0, "c0")
        fst_c1 = stage_chunk(f2t, L, "c1")
        for d in range(NDT):
            pj_cons_act(pj_mm(fst_f1, d, "f1"), e1q, d, 0)
        for d in range(NDT):
            pj_cons_act(pj_mm(fst_c0, d, "c0"), e2q, d, 0)
        for d in range(NDT):
            pj_cons_act(pj_mm(fst_c1, d, "c1"), e2q, d, L)

        # per-block DVE drain quotas (ops per projection-consumer point)
        quota = {0: 1, 1: 4, 2: 3}

        for bk in range(NCG):
            mm_slots = {}
            dv_slots = {}
            if bk < 3:
                c0, c1 = 2 * bk + 2, 2 * bk + 3
                # prefetch both chunks' staging DMAs at block start
                fsts = {c0: stage_chunk(f2t, c0 * L, f"c{c0}"),
                        c1: stage_chunk(f2t, c1 * L, f"c{c1}")}
                for idx, (cc, d) in enumerate([(c0, 0), (c0, 1), (c1, 0), (c1, 1)]):
                    m_at = (2, 4, 6, 7)[idx]

                    def mk(cc=cc, d=d, bk=bk):
                        def f():
                            ps = pj_mm(fsts[cc], d, f"c{cc}")
                            pj_cons_dve(ps, e2q, d, cc * L)
                            drain(quota[bk])
                        return f
                    mm_slots[m_at] = mk()
            else:
                # reduce-MMs right at block-3 start, rows two groups later:
                # everything evacuated by ~m5 so nothing lands in the tail
                for idx, (key, out_dram) in enumerate(red_list):
                    mm_slots[idx] = mk_reduce_mm(key)
                    dv_slots[idx + 2] = mk_row(key, out_dram)
            for m in range(8):
                sim_group(bk, m)
                if m in mm_slots:
                    mm_slots[m]()
                if m in dv_slots:
                    dv_slots[m]()
            if bk == 2:
                drain(len(dq))   # leftover banded work
                # blocks 0-2 stot columns ship now; the thin-line DMA cost
                # (128 descriptors) hides under block 3
                nc.sync.dma_start(
                    out=s_out_a[:],
                    in_=stot[:].rearrange("p (m c) -> p m c", c=NCG)[:, :, 0:3])

        nc.sync.dma_start(
            out=s_out_b[:],
            in_=stot[:].rearrange("p (m c) -> p m c", c=NCG)[:, :, 3])

    nc.compile()
    return nc


def kernel(feature1, feature2, W, b, positive_range_self, positive_range_tgt):
    r_self = int(np.asarray(positive_range_self))
    r_tgt = int(np.asarray(positive_range_tgt))
    assert 0 <= r_self <= PAD and 0 <= r_tgt <= PAD

    key = (r_self, r_tgt)
    if key not in _module_cache:
        _module_cache[key] = _build(r_self, r_tgt)
    nc = _module_cache[key]

    in_maps = _make_in_maps(feature1, feature2, W, b)
    res = bass_utils.run_bass_kernel_spmd(nc, in_maps, list(range(N_CORES)))

    # ---- host combine (fp64) ---------------------------------------------
    j = np.arange(L)
    loss_terms = []
    for i in range(N_CORES):
        r = res.results[i]
        Sa = r["s_out_a"].astype(np.float64).reshape(128, 8, 3).sum(axis=2)
        S = (Sa + r["s_out_b"].astype(np.float64)).T.reshape(L)
        t = SHIFT + np.log(S) - np.log(float(N))             # negative_j
        t -= r["pos_main"].astype(np.float64) / EXPA
        if r_self > 0:
            cnt = np.minimum(L - 1, j + r_self) - np.maximum(0, j - r_self) + 1.0
            t -= r["pos_self"].astype(np.float64) / EXPA / cnt
        if r_tgt > 0:
            cnt = np.minimum(L - 1, j + r_tgt) - np.maximum(0, j - r_tgt) + 1.0
            t -= r["pos_tgt"].astype(np.float64) / EXPA / cnt
        loss_terms.append(t)
    loss = np.mean(np.concatenate(loss_terms))
    return np.float32(loss)


def _make_in_maps(feature1, feature2, W, b):
    fp8 = ml_dtypes.float8_e4m3
    f1 = np.asarray(feature1, dtype=np.float32)
    f2 = np.asarray(feature2, dtype=np.float32)
    Wr = np.asarray(W, dtype=np.float32) * SQA
    bv = np.ascontiguousarray(np.asarray(b, dtype=np.float32) * SQA)

    # W: [DIN, DF] -> [128, KO*DF] partition-major (w[p, ko*DF+d] = W[ko*128+p, d])
    w_pm = np.ascontiguousarray(
        Wr.reshape(KO, 128, DF).transpose(1, 0, 2).reshape(128, KO * DF).astype(fp8))

    # f2: [B, L, DIN] -> [128, c, KO, L] chunk-contiguous partition-major
    f2T = f2.reshape(N, DIN).T.astype(fp8)                   # [DIN, N]
    f2_4d = (f2T.reshape(KO, 128, B, L).transpose(1, 2, 0, 3))   # [p, c, ko, n]
    in_maps = []
    for i in range(N_CORES):
        f1T = f1[i].T.astype(fp8)                            # [DIN, L]
        f1_pm = np.ascontiguousarray(
            f1T.reshape(KO, 128, L).transpose(1, 0, 2).reshape(128, KO * L))
        f2_rot = np.ascontiguousarray(
            np.roll(f2_4d, -i, axis=1).reshape(128, B * KO * L))
        in_maps.append({"f1t": f1_pm, "f2t": f2_rot, "w_in": w_pm, "b_in": bv})
    return in_maps


# revision 27
# speedup vs baseline: 1.0629x; 1.0571x over previous
"""Contrastive-learning NCE loss on 8 trn2 NeuronCores (Bass/Tile).

Problem (hardcoded shapes): B=8, L=1024, D_in=512, D_feat=256, N=B*L=8192.
  emb_k = relu(feature_k @ W + b)                     [B, L, Df]
  positive = <e1,e2> + banded_diag_mean terms         [N]
  negative = logsumexp(e1 @ e2.T, axis=-1) - log(N)   [N]
  loss = mean(-positive + negative)

Sharding: token dim N split across 8 cores = one batch row each (L == N/8).
Each core computes its [1024, 8192] slab of the similarity matrix against the
full emb_2 (recomputed locally from full feature2). The host rotates feature2
per core (chunk-granular) so the core's own batch sits at columns 0:1023 ->
the SPMD program is core-index free.

v4 design (baseline bf16 ~160-187us -> v2 fp8 126us -> v3 119us):
  * All matmuls fp8e4m3 + DoubleRow (K=256 per MM; ~220ns per 512-col MM).
  * W (and b) are pre-scaled by sqrt(A), A = 128/ln2, so the sim PSUM holds
    A*sim: the ScalarE exp uses the free affine (scale=1/A, bias=-64), and
    offloaded groups can run a 2-op Schraudolph exp on the DVE:
      int16(max(ps + B2, 0)) bitcast bf16 == exp(sim-64) to ~1.5%,
    then tensor_reduce -> the row-sum. 4 of 32 groups go to the (otherwise
    slack) DVE, relieving the ACT roofline. Banded terms come out scaled by
    A; the host divides them back.
  * Constant logsumexp shift (-64) is exact for any shift; max sim ~120 keeps
    exp in fp32 range. exp is computed in-place on PSUM with the fused row
    accumulator; host adds 64 + log(S).
  * Inputs are staged host-side into partition-major chunk-contiguous
    layouts so every DMA moves 4KB per partition-line (128 descriptors vs
    512 thin ones) -- the first projection starts ~5us earlier.
  * One software pipeline paced by ACT: the 2x[128,2048] PSUM ring is shared
    by sim groups and projection tiles, with projection d-tiles interleaved
    INSIDE the sim m-loop. Prologue projection consumers run on the ACT
    (Relu lives in the same table set as Exp), leaving the DVE free.
  * Banded-term DVE work is queued as single-op closures drained on a
    per-block quota at the projection-consumer points; the tiny PE
    reduce-MMs slot into block 3.
  * A PE warmup burst during the DMA head defeats the HAM cold-clock.
"""

import numpy as np
import ml_dtypes
from collections import deque
from contextlib import ExitStack

import concourse.bass as bass
import concourse.tile as tile
from concourse import bacc, mybir
from concourse import bass_utils

dt = mybir.dt
AF = mybir.ActivationFunctionType
ALU = mybir.AluOpType
PM = mybir.MatmulPerfMode

N_CORES = 8
B, L, DIN, DF = 8, 1024, 512, 256
N = B * L
KO = DIN // 128     # 4 k-tiles of the projection contraction
NDT = DF // 128     # 2 d-tiles of the embedding dim
PAD = 4             # box-filter padding (max supported positive_range)
LP = L + 2 * PAD
CW = 2048           # sim-phase column group (one PSUM tile / one ACT)
NCG = N // CW       # 4 sim blocks
SHIFT = 64.0        # constant logsumexp shift

EXPA = 128.0 / np.log(2.0)          # 184.665 = Schraudolph slope
SQA = float(np.sqrt(EXPA))          # weight pre-scale
SIGMA = 8.13                        # Schraudolph bias tuning (HW-calibrated)
B2 = 16256.0 - SIGMA - EXPA * SHIFT
OFFLOAD = {(0, 0), (2, 0), (3, 0), (3, 4)}   # (block, m) exp groups on the DVE

_module_cache = {}


def _box_terms(w: int):
    """Decompose window width w (odd, <= 2*PAD+1) into power-of-2 segments."""
    terms, off = [], 0
    for p in (8, 4, 2, 1):
        if w >= p:
            terms.append((p, off))
            off += p
            w -= p
    assert w == 0
    return terms


def _build(r_self: int, r_tgt: int):
    nc = bacc.Bacc("TRN2", target_bir_lowering=False, debug=False, num_devices=N_CORES)

    # partition-major staged inputs (see _make_in_maps)
    f1t = nc.dram_tensor("f1t", [128, KO * L], dt.float8e4, kind="ExternalInput").ap()
    f2t = nc.dram_tensor("f2t", [128, B * KO * L], dt.float8e4, kind="ExternalInput").ap()
    w_in = nc.dram_tensor("w_in", [128, KO * DF], dt.float8e4, kind="ExternalInput").ap()
    b_in = nc.dram_tensor("b_in", [128, NDT], dt.float32, kind="ExternalInput").ap()

    pos_main = nc.dram_tensor("pos_main", [L], dt.float32, kind="ExternalOutput").ap()
    pos_self = nc.dram_tensor("pos_self", [L], dt.float32, kind="ExternalOutput").ap()
    pos_tgt = nc.dram_tensor("pos_tgt", [L], dt.float32, kind="ExternalOutput").ap()
    s_out_a = nc.dram_tensor("s_out_a", [128, 8 * 3], dt.float32, kind="ExternalOutput").ap()
    s_out_b = nc.dram_tensor("s_out_b", [128, 8], dt.float32, kind="ExternalOutput").ap()

    with tile.TileContext(nc) as tc, ExitStack() as ctx:
        const = ctx.enter_context(tc.tile_pool(name="const", bufs=1))
        stage = ctx.enter_context(tc.tile_pool(name="stage", bufs=3))
        emb = ctx.enter_context(tc.tile_pool(name="emb", bufs=1))
        band = ctx.enter_context(tc.tile_pool(name="band", bufs=1))
        prodp = ctx.enter_context(tc.tile_pool(name="prodp", bufs=8))
        rows = ctx.enter_context(tc.tile_pool(name="rows", bufs=1))
        mmp = ctx.enter_context(tc.tile_pool(name="mmp", bufs=2, space="PSUM"))

        # ---- constants -------------------------------------------------
        wt = const.tile([128, KO, DF], dt.float8e4)    # W[ko*128+p, d] at [p, ko, d]
        nc.sync.dma_start(out=wt[:], in_=w_in[:].rearrange("p (ko d) -> p ko d", ko=KO))
        b_col = const.tile([128, NDT], dt.float32)     # bias per (d%128, dtile)
        ones_f = const.tile([128, 1], dt.float32)
        nc.vector.memset(ones_f[:], 1.0)
        ones = const.tile([128, 1], dt.bfloat16)
        nc.vector.tensor_copy(ones[:], ones_f[:])
        neg_shift = const.tile([128, 1], dt.float32)
        nc.vector.memset(neg_shift[:], -SHIFT)
        warm = const.tile([128, 1], dt.float32)
        # dummy exp: the ACT exp-table load happens during the DMA head
        nc.scalar.activation(warm[:], ones_f[:], AF.Exp, bias=neg_shift[:], scale=1.0)

        # PE warmup: junk matmuls during the DMA head keep the HAM activity
        # monitor busy so real MMs run at 2.4GHz from the start
        wst = const.tile([128, 128], dt.bfloat16)
        wmv = const.tile([128, 512], dt.bfloat16)
        nc.vector.memset(wst[:], 0.25)
        nc.vector.memset(wmv[:], 0.25)

        # ---- embeddings (all scaled by sqrt(A)) ------------------------
        e1q = emb.tile([128, NDT, L], dt.float8e4, name="e1q", tag="e1q")
        e2q = emb.tile([128, NDT, N], dt.float8e4, name="e2q", tag="e2q")
        e1b = emb.tile([128, NDT, LP], dt.bfloat16, name="e1b", tag="e1b")
        e2b = emb.tile([128, NDT, LP], dt.bfloat16, name="e2b", tag="e2b")
        nc.vector.memzero(e1b[:])
        nc.vector.memzero(e2b[:])

        stot = const.tile([128, 8 * NCG], dt.float32)   # [p, bk*8 + m] (block-major)
        t16 = const.tile([128, CW], dt.int16)           # Schraudolph scratch

        # ---- projection pieces -----------------------------------------
        def pj_mm(src_view, d, tag):
            """PE half of a projection d-tile (chunk already staged)."""
            ps = mmp.tile([128, L], dt.float32, tag="mm", name=f"pj_{tag}_{d}")
            for kk in range(KO // 2):
                for h in range(L // 512):
                    nc.tensor.matmul(
                        ps[:, h * 512:(h + 1) * 512],
                        wt[:, 2 * kk:2 * kk + 2, d * 128:(d + 1) * 128],
                        src_view[:, 2 * kk:2 * kk + 2, h * 512:(h + 1) * 512],
                        start=(kk == 0), stop=(kk == KO // 2 - 1),
                        perf_mode=PM.DoubleRow)
            return ps

        def stage_chunk(src_ap, col0, tag):
            fst = stage.tile([128, KO, L], dt.float8e4, tag="fstage", name=f"fst_{tag}")
            nc.sync.dma_start(
                out=fst[:],
                in_=src_ap[:, col0 * KO:(col0 + L) * KO]
                    .rearrange("p (ko n) -> p ko n", ko=KO))
            return fst

        def pj_cons_dve(ps, q_dst, d, q_col0):
            nc.vector.tensor_scalar(
                q_dst[:, d, q_col0:q_col0 + L], ps[:],
                b_col[:, d:d + 1], 0.0, ALU.add, ALU.max)

        def pj_cons_act(ps, q_dst, d, q_col0):
            nc.scalar.activation(q_dst[:, d, q_col0:q_col0 + L], ps[:],
                                 AF.Relu, bias=b_col[:, d:d + 1], scale=1.0)

        # ---- sim group --------------------------------------------------
        def sim_group(bk, m):
            ps = mmp.tile([128, CW], dt.float32, tag="mm", name=f"sim_{bk}_{m}")
            for q in range(CW // 512):
                nc.tensor.matmul(
                    ps[:, q * 512:(q + 1) * 512],
                    e1q[:, :, m * 128:(m + 1) * 128],
                    e2q[:, :, bk * CW + q * 512: bk * CW + (q + 1) * 512],
                    start=True, stop=True, perf_mode=PM.DoubleRow)
            col = stot[:, bk * 8 + m: bk * 8 + m + 1]
            if (bk, m) in OFFLOAD:
                # DVE Schraudolph: exp(sim-64) ~= bf16_bits(A*sim + B2)
                nc.vector.tensor_scalar(t16[:], ps[:], B2, 0.0, ALU.add, ALU.max)
                nc.vector.tensor_reduce(col, t16[:].bitcast(dt.bfloat16),
                                        mybir.AxisListType.X, ALU.add)
            else:
                nc.scalar.activation(ps[:], ps[:], AF.Exp,
                                     bias=neg_shift[:], scale=1.0 / EXPA,
                                     accum_out=col)

        # ---- banded-term closures (drained on per-block quotas) ---------
        boxes = {}
        prods = {"main": [], "self": [], "tgt": []}
        dq = deque()

        def mk_copy(dst, src_q, d):
            def f():
                nc.vector.tensor_copy(dst[:, d, PAD:PAD + L], src_q[:, d, 0:L])
            return f

        def queue_boxsum(key, src, d, r):
            """Queue the box-filter as single-op closures; stores result view."""
            wdt = 2 * r + 1
            state = {1: src[:, d, :]}

            def mk_shift(p):
                def f():
                    sp = band.tile([128, LP], dt.bfloat16, name=f"s{p}_{key}_{d}",
                                   tag=f"s{p}")
                    h = p // 2
                    nv = LP - p + 1
                    nc.vector.tensor_tensor(sp[:, :nv], state[h][:, :nv],
                                            state[h][:, h:h + nv], ALU.add)
                    state[p] = sp
                return f
            for p in (2, 4, 8):
                if wdt >= p:
                    dq.append(mk_shift(p))

            def mk_fin():
                def f():
                    terms = _box_terms(wdt)
                    t0 = PAD - r
                    if len(terms) == 1:
                        p0, o0 = terms[0]
                        boxes[(key, d)] = state[p0][:, t0 + o0: t0 + o0 + L]
                        return
                    acc = band.tile([128, L], dt.bfloat16, name=f"box_{key}_{d}",
                                    tag="box", bufs=6)
                    p0, o0 = terms[0]
                    p1, o1 = terms[1]
                    nc.vector.tensor_tensor(acc[:], state[p0][:, t0 + o0: t0 + o0 + L],
                                            state[p1][:, t0 + o1: t0 + o1 + L], ALU.add)
                    for p, o in terms[2:]:
                        nc.vector.tensor_tensor(acc[:], acc[:],
                                                state[p][:, t0 + o: t0 + o + L], ALU.add)
                    boxes[(key, d)] = acc[:]
                return f
            dq.append(mk_fin())

        def mk_prod(key, gi, a_fn, b_fn):
            def f():
                prod = prodp.tile([128, L], dt.bfloat16, tag=f"prod_{key}_{gi}", bufs=1)
                nc.vector.tensor_tensor(prod[:], a_fn(), b_fn(), ALU.mult)
                prods[key].append(prod)
            return f

        e1v = [e1b[:, d, PAD:PAD + L] for d in range(NDT)]
        e2v = [e2b[:, d, PAD:PAD + L] for d in range(NDT)]

        for d in range(NDT):
            dq.append(mk_copy(e1b, e1q, d))
        for d in range(NDT):
            dq.append(mk_copy(e2b, e2q, d))
        if r_self:
            for d in range(NDT):
                queue_boxsum("bx1", e1b, d, r_self)
            for d in range(NDT):
                queue_boxsum("bx2", e2b, d, r_self)
        if r_tgt and r_tgt != r_self:
            for d in range(NDT):
                queue_boxsum("bxt", e2b, d, r_tgt)
        tkey = "bxt" if (r_tgt and r_tgt != r_self) else "bx2"
        for d in range(NDT):
            dq.append(mk_prod("main", d, lambda d=d: e1v[d], lambda d=d: e2v[d]))
        if r_self:
            for d in range(NDT):
                dq.append(mk_prod("self", d, lambda d=d: e1v[d],
                                  lambda d=d: boxes[("bx1", d)]))
            for d in range(NDT):
                dq.append(mk_prod("self", NDT + d, lambda d=d: e2v[d],
                                  lambda d=d: boxes[("bx2", d)]))
        if r_tgt:
            for d in range(NDT):
                dq.append(mk_prod("tgt", d, lambda d=d: e1v[d],
                                  lambda d=d: boxes[(tkey, d)]))

        def drain(k):
            for _ in range(k):
                if dq:
                    dq.popleft()()

        # ---- pos reduce-MMs + row evacuation (block 3) -------------------
        rps = {}
        combined = {}

        def mk_combine(key):
            """Elementwise-sum the group's prods so the PE reduce is 2 MMs."""
            def f():
                pr = prods[key]
                if len(pr) == 1:
                    combined[key] = pr[0]
                    return
                acc = prodp.tile([128, L], dt.bfloat16, tag=f"comb_{key}", bufs=1)
                nc.vector.tensor_tensor(acc[:], pr[0][:], pr[1][:], ALU.add)
                for p in pr[2:]:
                    nc.vector.tensor_tensor(acc[:], acc[:], p[:], ALU.add)
                combined[key] = acc
            return f

        def mk_reduce_mm(key):
            # matmul out must fit one PSUM bank (512 fp32) -> two 512 halves
            def f():
                rp = mmp.tile([1, L], dt.float32, tag="mm", name=f"rp_{key}")
                cp = combined[key]
                for h in range(L // 512):
                    nc.tensor.matmul(rp[:, h * 512:(h + 1) * 512], ones[:],
                                     cp[:, h * 512:(h + 1) * 512],
                                     start=True, stop=True)
                rps[key] = rp
            return f

        def mk_row(key, out_dram):
            def f():
                row = rows.tile([1, L], dt.float32, tag=f"row_{key}")
                nc.vector.tensor_copy(row[:], rps[key][:])
                nc.sync.dma_start(out=out_dram[:].rearrange("(one n) -> one n", one=1),
                                  in_=row[:])
            return f

        def zero_out(out_dram, tag):
            zr = rows.tile([1, L], dt.float32, tag=f"zr_{tag}")
            nc.vector.memset(zr[:], 0.0)
            nc.sync.dma_start(out=out_dram[:].rearrange("(one n) -> one n", one=1),
                              in_=zr[:])

        red_list = [("main", pos_main)]
        if r_self:
            red_list.append(("self", pos_self))
        else:
            zero_out(pos_self, "self")
        if r_tgt:
            red_list.append(("tgt", pos_tgt))
        else:
            zero_out(pos_tgt, "tgt")
        for key, _ in red_list:
            dq.append(mk_combine(key))

        # ---- schedule ---------------------------------------------------
        for w in range(2):
            wps = mmp.tile([128, 512], dt.float32, tag="mm", name=f"wps_{w}")
            for _ in range(4):
                nc.tensor.matmul(wps[:], wst[:], wmv[:], start=True, stop=True)

        # prologue: f1 + e2 chunks 0,1; relu consumers on ACT (same table set)
        fst_f1 = stage_chunk(f1t, 0, "f1")
        fst_c0 = stage_chunk(f2t, 0, "c0")
        fst_c1 = stage_chunk(f2t, L, "c1")
        nc.sync.dma_start(out=b_col[:], in_=b_in[:])
        for d in range(NDT):
            pj_cons_act(pj_mm(fst_f1, d, "f1"), e1q, d, 0)
        for d in range(NDT):
            pj_cons_act(pj_mm(fst_c0, d, "c0"), e2q, d, 0)
        for d in range(NDT):
            pj_cons_act(pj_mm(fst_c1, d, "c1"), e2q, d, L)

        # per-block DVE drain quotas (ops per projection-consumer point)
        quota = {0: 1, 1: 4, 2: 3}

        for bk in range(NCG):
            mm_slots = {}
            dv_slots = {}
            if bk < 3:
                c0, c1 = 2 * bk + 2, 2 * bk + 3
                # prefetch both chunks' staging DMAs at block start
                fsts = {c0: stage_chunk(f2t, c0 * L, f"c{c0}"),
                        c1: stage_chunk(f2t, c1 * L, f"c{c1}")}
                for idx, (cc, d) in enumerate([(c0, 0), (c0, 1), (c1, 0), (c1, 1)]):
                    m_at = (2, 4, 6, 7)[idx]

                    def mk(cc=cc, d=d, bk=bk):
                        def f():
                            ps = pj_mm(fsts[cc], d, f"c{cc}")
                            pj_cons_dve(ps, e2q, d, cc * L)
                            drain(quota[bk])
                        return f
                    mm_slots[m_at] = mk()
            else:
                # reduce-MMs right at block-3 start, rows two groups later:
                # everything evacuated by ~m5 so nothing lands in the tail
                for idx, (key, out_dram) in enumerate(red_list):
                    mm_slots[idx] = mk_reduce_mm(key)
                    dv_slots[idx + 2] = mk_row(key, out_dram)
            for m in range(8):
                sim_group(bk, m)
                if m in mm_slots:
                    mm_slots[m]()
                if m in dv_slots:
                    dv_slots[m]()
            if bk == 2:
                drain(len(dq))   # leftover banded work
                # blocks 0-2 stot columns ship now; the thin-line DMA cost
                # (128 descriptors) hides under block 3
                nc.sync.dma_start(out=s_out_a[:], in_=stot[:, 0:24])

        nc.sync.dma_start(out=s_out_b[:], in_=stot[:, 24:32])

    nc.compile()
    return nc


def kernel(feature1, feature2, W, b, positive_range_self, positive_range_tgt):
    r_self = int(np.asarray(positive_range_self))
    r_tgt = int(np.asarray(positive_range_tgt))
    assert 0 <= r_self <= PAD and 0 <= r_tgt <= PAD

    key = (r_self, r_tgt)
    if key not in _module_cache:
        _module_cache[key] = _build(r_self, r_tgt)
    nc = _module_cache[key]

    in_maps = _make_in_maps(feature1, feature2, W, b)
    res = bass_utils.run_bass_kernel_spmd(nc, in_maps, list(range(N_CORES)))

    # ---- host combine (fp64) ---------------------------------------------
    j = np.arange(L)
    loss_terms = []
    for i in range(N_CORES):
        r = res.results[i]
        Sa = r["s_out_a"].astype(np.float64).reshape(128, 3, 8).sum(axis=1)
        S = (Sa + r["s_out_b"].astype(np.float64)).T.reshape(L)
        t = SHIFT + np.log(S) - np.log(float(N))             # negative_j
        t -= r["pos_main"].astype(np.float64) / EXPA
        if r_self > 0:
            cnt = np.minimum(L - 1, j + r_self) - np.maximum(0, j - r_self) + 1.0
            t -= r["pos_self"].astype(np.float64) / EXPA / cnt
        if r_tgt > 0:
            cnt = np.minimum(L - 1, j + r_tgt) - np.maximum(0, j - r_tgt) + 1.0
            t -= r["pos_tgt"].astype(np.float64) / EXPA / cnt
        loss_terms.append(t)
    loss = np.mean(np.concatenate(loss_terms))
    return np.float32(loss)


def _make_in_maps(feature1, feature2, W, b):
    fp8 = ml_dtypes.float8_e4m3
    f1 = np.asarray(feature1, dtype=np.float32)
    f2 = np.asarray(feature2, dtype=np.float32)
    Wr = np.asarray(W, dtype=np.float32) * SQA
    bv0 = np.asarray(b, dtype=np.float32) * SQA
    bv = np.ascontiguousarray(bv0.reshape(NDT, 128).T)      # [128, NDT]

    # W: [DIN, DF] -> [128, KO*DF] partition-major (w[p, ko*DF+d] = W[ko*128+p, d])
    w_pm = np.ascontiguousarray(
        Wr.reshape(KO, 128, DF).transpose(1, 0, 2).reshape(128, KO * DF).astype(fp8))

    # f2: [B, L, DIN] -> [128, c, KO, L] chunk-contiguous partition-major
    f2T = f2.reshape(N, DIN).T.astype(fp8)                   # [DIN, N]
    f2_4d = (f2T.reshape(KO, 128, B, L).transpose(1, 2, 0, 3))   # [p, c, ko, n]
    in_maps = []
    for i in range(N_CORES):
        f1T = f1[i].T.astype(fp8)                            # [DIN, L]
        f1_pm = np.ascontiguousarray(
            f1T.reshape(KO, 128, L).transpose(1, 0, 2).reshape(128, KO * L))
        f2_rot = np.ascontiguousarray(
            np.roll(f2_4d, -i, axis=1).reshape(128, B * KO * L))
        in_maps.append({"f1t": f1_pm, "f2t": f2_rot, "w_in": w_pm, "b_in": bv})
    return in_maps


# revision 28
# speedup vs baseline: 1.0914x; 1.0268x over previous
"""Contrastive-learning NCE loss on 8 trn2 NeuronCores (Bass/Tile).

Problem (hardcoded shapes): B=8, L=1024, D_in=512, D_feat=256, N=B*L=8192.
  emb_k = relu(feature_k @ W + b)                     [B, L, Df]
  positive = <e1,e2> + banded_diag_mean terms         [N]
  negative = logsumexp(e1 @ e2.T, axis=-1) - log(N)   [N]
  loss = mean(-positive + negative)

Sharding: token dim N split across 8 cores = one batch row each (L == N/8).
Each core computes its [1024, 8192] slab of the similarity matrix against the
full emb_2 (recomputed locally from full feature2). The host rotates feature2
per core (chunk-granular) so the core's own batch sits at columns 0:1023 ->
the SPMD program is core-index free.

v4 design (baseline bf16 ~160-187us -> v2 fp8 126us -> v3 119us):
  * All matmuls fp8e4m3 + DoubleRow (K=256 per MM; ~220ns per 512-col MM).
  * W (and b) are pre-scaled by sqrt(A), A = 128/ln2, so the sim PSUM holds
    A*sim: the ScalarE exp uses the free affine (scale=1/A, bias=-64), and
    offloaded groups can run a 2-op Schraudolph exp on the DVE:
      int16(max(ps + B2, 0)) bitcast bf16 == exp(sim-64) to ~1.5%,
    then tensor_reduce -> the row-sum. 4 of 32 groups go to the (otherwise
    slack) DVE, relieving the ACT roofline. Banded terms come out scaled by
    A; the host divides them back.
  * Constant logsumexp shift (-64) is exact for any shift; max sim ~120 keeps
    exp in fp32 range. exp is computed in-place on PSUM with the fused row
    accumulator; host adds 64 + log(S).
  * Inputs are staged host-side into partition-major chunk-contiguous
    layouts so every DMA moves 4KB per partition-line (128 descriptors vs
    512 thin ones) -- the first projection starts ~5us earlier.
  * One software pipeline paced by ACT: the 2x[128,2048] PSUM ring is shared
    by sim groups and projection tiles, with projection d-tiles interleaved
    INSIDE the sim m-loop. Prologue projection consumers run on the ACT
    (Relu lives in the same table set as Exp), leaving the DVE free.
  * Banded-term DVE work is queued as single-op closures drained on a
    per-block quota at the projection-consumer points; the tiny PE
    reduce-MMs slot into block 3.
  * A PE warmup burst during the DMA head defeats the HAM cold-clock.
"""

import numpy as np
import ml_dtypes
from collections import deque
from contextlib import ExitStack

import concourse.bass as bass
import concourse.tile as tile
from concourse import bacc, mybir
from concourse import bass_utils

dt = mybir.dt
AF = mybir.ActivationFunctionType
ALU = mybir.AluOpType
PM = mybir.MatmulPerfMode

N_CORES = 8
B, L, DIN, DF = 8, 1024, 512, 256
N = B * L
KO = DIN // 128     # 4 k-tiles of the projection contraction
NDT = DF // 128     # 2 d-tiles of the embedding dim
PAD = 4             # box-filter padding (max supported positive_range)
LP = L + 2 * PAD
CW = 2048           # sim-phase column group (one PSUM tile / one ACT)
NCG = N // CW       # 4 sim blocks
SHIFT = 64.0        # constant logsumexp shift

EXPA = 128.0 / np.log(2.0)          # 184.665 = Schraudolph slope
SQA = float(np.sqrt(EXPA))          # weight pre-scale
SIGMA = 8.13                        # Schraudolph bias tuning (HW-calibrated)
B2 = 16256.0 - SIGMA - EXPA * SHIFT
OFFLOAD = {(0, 0), (2, 0), (3, 0), (3, 4)}   # (block, m) exp groups on the DVE

_module_cache = {}


def _box_terms(w: int):
    """Decompose window width w (odd, <= 2*PAD+1) into power-of-2 segments."""
    terms, off = [], 0
    for p in (8, 4, 2, 1):
        if w >= p:
            terms.append((p, off))
            off += p
            w -= p
    assert w == 0
    return terms


def _build(r_self: int, r_tgt: int):
    nc = bacc.Bacc("TRN2", target_bir_lowering=False, debug=False, num_devices=N_CORES)

    # partition-major staged inputs (see _make_in_maps)
    f1t = nc.dram_tensor("f1t", [128, KO * L], dt.float8e4, kind="ExternalInput").ap()
    f2t = nc.dram_tensor("f2t", [128, B * KO * L], dt.float8e4, kind="ExternalInput").ap()
    w_in = nc.dram_tensor("w_in", [128, KO * DF], dt.float8e4, kind="ExternalInput").ap()
    b_in = nc.dram_tensor("b_in", [128, NDT], dt.float32, kind="ExternalInput").ap()

    pos_main = nc.dram_tensor("pos_main", [L], dt.float32, kind="ExternalOutput").ap()
    pos_self = nc.dram_tensor("pos_self", [L], dt.float32, kind="ExternalOutput").ap()
    pos_tgt = nc.dram_tensor("pos_tgt", [L], dt.float32, kind="ExternalOutput").ap()
    s_out_a = nc.dram_tensor("s_out_a", [128, 8 * 3], dt.float32, kind="ExternalOutput").ap()
    s_out_b = nc.dram_tensor("s_out_b", [128, 8], dt.float32, kind="ExternalOutput").ap()

    with tile.TileContext(nc) as tc, ExitStack() as ctx:
        const = ctx.enter_context(tc.tile_pool(name="const", bufs=1))
        stage = ctx.enter_context(tc.tile_pool(name="stage", bufs=3))
        emb = ctx.enter_context(tc.tile_pool(name="emb", bufs=1))
        band = ctx.enter_context(tc.tile_pool(name="band", bufs=1))
        prodp = ctx.enter_context(tc.tile_pool(name="prodp", bufs=8))
        rows = ctx.enter_context(tc.tile_pool(name="rows", bufs=1))
        mmp = ctx.enter_context(tc.tile_pool(name="mmp", bufs=2, space="PSUM"))

        # ---- constants -------------------------------------------------
        wt = const.tile([128, KO, DF], dt.float8e4)    # W[ko*128+p, d] at [p, ko, d]
        nc.sync.dma_start(out=wt[:], in_=w_in[:].rearrange("p (ko d) -> p ko d", ko=KO))
        b_col = const.tile([128, NDT], dt.float32)     # bias per (d%128, dtile)
        nc.sync.dma_start(out=b_col[:], in_=b_in[:])
        ones_f = const.tile([128, 1], dt.float32)
        nc.vector.memset(ones_f[:], 1.0)
        ones = const.tile([128, 1], dt.bfloat16)
        nc.vector.tensor_copy(ones[:], ones_f[:])
        neg_shift = const.tile([128, 1], dt.float32)
        nc.vector.memset(neg_shift[:], -SHIFT)
        warm = const.tile([128, 1], dt.float32)
        # dummy exp: the ACT exp-table load happens during the DMA head
        nc.scalar.activation(warm[:], ones_f[:], AF.Exp, bias=neg_shift[:], scale=1.0)

        # PE warmup: junk matmuls during the DMA head keep the HAM activity
        # monitor busy so real MMs run at 2.4GHz from the start
        wst = const.tile([128, 128], dt.bfloat16)
        wmv = const.tile([128, 512], dt.bfloat16)
        nc.vector.memset(wst[:], 0.25)
        nc.vector.memset(wmv[:], 0.25)

        # ---- embeddings (all scaled by sqrt(A)) ------------------------
        e1q = emb.tile([128, NDT, L], dt.float8e4, name="e1q", tag="e1q")
        e2q = emb.tile([128, NDT, N], dt.float8e4, name="e2q", tag="e2q")
        e1b = emb.tile([128, NDT, LP], dt.bfloat16, name="e1b", tag="e1b")
        e2b = emb.tile([128, NDT, LP], dt.bfloat16, name="e2b", tag="e2b")
        nc.vector.memzero(e1b[:])
        nc.vector.memzero(e2b[:])

        stot = const.tile([128, 8 * NCG], dt.float32)   # [p, bk*8 + m] (block-major)
        t16 = const.tile([128, CW], dt.int16)           # Schraudolph scratch

        # ---- projection pieces -----------------------------------------
        def pj_mm(src_view, d, tag):
            """PE half of a projection d-tile (chunk already staged)."""
            ps = mmp.tile([128, L], dt.float32, tag="mm", name=f"pj_{tag}_{d}")
            for kk in range(KO // 2):
                for h in range(L // 512):
                    nc.tensor.matmul(
                        ps[:, h * 512:(h + 1) * 512],
                        wt[:, 2 * kk:2 * kk + 2, d * 128:(d + 1) * 128],
                        src_view[:, 2 * kk:2 * kk + 2, h * 512:(h + 1) * 512],
                        start=(kk == 0), stop=(kk == KO // 2 - 1),
                        perf_mode=PM.DoubleRow)
            return ps

        def stage_chunk(src_ap, col0, tag):
            fst = stage.tile([128, KO, L], dt.float8e4, tag="fstage", name=f"fst_{tag}")
            nc.sync.dma_start(
                out=fst[:],
                in_=src_ap[:, col0 * KO:(col0 + L) * KO]
                    .rearrange("p (ko n) -> p ko n", ko=KO))
            return fst

        def pj_cons_dve(ps, q_dst, d, q_col0):
            nc.vector.tensor_scalar(
                q_dst[:, d, q_col0:q_col0 + L], ps[:],
                b_col[:, d:d + 1], 0.0, ALU.add, ALU.max)

        def pj_cons_act(ps, q_dst, d, q_col0):
            nc.scalar.activation(q_dst[:, d, q_col0:q_col0 + L], ps[:],
                                 AF.Relu, bias=b_col[:, d:d + 1], scale=1.0)

        # ---- sim group --------------------------------------------------
        def sim_group(bk, m):
            ps = mmp.tile([128, CW], dt.float32, tag="mm", name=f"sim_{bk}_{m}")
            for q in range(CW // 512):
                nc.tensor.matmul(
                    ps[:, q * 512:(q + 1) * 512],
                    e1q[:, :, m * 128:(m + 1) * 128],
                    e2q[:, :, bk * CW + q * 512: bk * CW + (q + 1) * 512],
                    start=True, stop=True, perf_mode=PM.DoubleRow)
            col = stot[:, bk * 8 + m: bk * 8 + m + 1]
            if (bk, m) in OFFLOAD:
                # DVE Schraudolph: exp(sim-64) ~= bf16_bits(A*sim + B2)
                nc.vector.tensor_scalar(t16[:], ps[:], B2, 0.0, ALU.add, ALU.max)
                nc.vector.tensor_reduce(col, t16[:].bitcast(dt.bfloat16),
                                        mybir.AxisListType.X, ALU.add)
            else:
                nc.scalar.activation(ps[:], ps[:], AF.Exp,
                                     bias=neg_shift[:], scale=1.0 / EXPA,
                                     accum_out=col)

        # ---- banded-term closures (drained on per-block quotas) ---------
        boxes = {}
        prods = {"main": [], "self": [], "tgt": []}
        dq = deque()

        def mk_copy(dst, src_q, d):
            def f():
                nc.vector.tensor_copy(dst[:, d, PAD:PAD + L], src_q[:, d, 0:L])
            return f

        def queue_boxsum(key, src, d, r):
            """Queue the box-filter as single-op closures; stores result view."""
            wdt = 2 * r + 1
            state = {1: src[:, d, :]}

            def mk_shift(p):
                def f():
                    sp = band.tile([128, LP], dt.bfloat16, name=f"s{p}_{key}_{d}",
                                   tag=f"s{p}")
                    h = p // 2
                    nv = LP - p + 1
                    nc.vector.tensor_tensor(sp[:, :nv], state[h][:, :nv],
                                            state[h][:, h:h + nv], ALU.add)
                    state[p] = sp
                return f
            for p in (2, 4, 8):
                if wdt >= p:
                    dq.append(mk_shift(p))

            def mk_fin():
                def f():
                    terms = _box_terms(wdt)
                    t0 = PAD - r
                    if len(terms) == 1:
                        p0, o0 = terms[0]
                        boxes[(key, d)] = state[p0][:, t0 + o0: t0 + o0 + L]
                        return
                    acc = band.tile([128, L], dt.bfloat16, name=f"box_{key}_{d}",
                                    tag="box", bufs=6)
                    p0, o0 = terms[0]
                    p1, o1 = terms[1]
                    nc.vector.tensor_tensor(acc[:], state[p0][:, t0 + o0: t0 + o0 + L],
                                            state[p1][:, t0 + o1: t0 + o1 + L], ALU.add)
                    for p, o in terms[2:]:
                        nc.vector.tensor_tensor(acc[:], acc[:],
                                                state[p][:, t0 + o: t0 + o + L], ALU.add)
                    boxes[(key, d)] = acc[:]
                return f
            dq.append(mk_fin())

        def mk_prod(key, gi, a_fn, b_fn):
            def f():
                prod = prodp.tile([128, L], dt.bfloat16, tag=f"prod_{key}_{gi}", bufs=1)
                nc.vector.tensor_tensor(prod[:], a_fn(), b_fn(), ALU.mult)
                prods[key].append(prod)
            return f

        e1v = [e1b[:, d, PAD:PAD + L] for d in range(NDT)]
        e2v = [e2b[:, d, PAD:PAD + L] for d in range(NDT)]

        for d in range(NDT):
            dq.append(mk_copy(e1b, e1q, d))
        for d in range(NDT):
            dq.append(mk_copy(e2b, e2q, d))
        if r_self:
            for d in range(NDT):
                queue_boxsum("bx1", e1b, d, r_self)
            for d in range(NDT):
                queue_boxsum("bx2", e2b, d, r_self)
        if r_tgt and r_tgt != r_self:
            for d in range(NDT):
                queue_boxsum("bxt", e2b, d, r_tgt)
        tkey = "bxt" if (r_tgt and r_tgt != r_self) else "bx2"
        for d in range(NDT):
            dq.append(mk_prod("main", d, lambda d=d: e1v[d], lambda d=d: e2v[d]))
        if r_self:
            for d in range(NDT):
                dq.append(mk_prod("self", d, lambda d=d: e1v[d],
                                  lambda d=d: boxes[("bx1", d)]))
            for d in range(NDT):
                dq.append(mk_prod("self", NDT + d, lambda d=d: e2v[d],
                                  lambda d=d: boxes[("bx2", d)]))
        if r_tgt:
            for d in range(NDT):
                dq.append(mk_prod("tgt", d, lambda d=d: e1v[d],
                                  lambda d=d: boxes[(tkey, d)]))

        def drain(k):
            for _ in range(k):
                if dq:
                    dq.popleft()()

        # ---- pos reduce-MMs + row evacuation (block 3) -------------------
        rps = {}
        combined = {}

        def mk_combine(key):
            """Elementwise-sum the group's prods so the PE reduce is 2 MMs."""
            def f():
                pr = prods[key]
                if len(pr) == 1:
                    combined[key] = pr[0]
                    return
                acc = prodp.tile([128, L], dt.bfloat16, tag=f"comb_{key}", bufs=1)
                nc.vector.tensor_tensor(acc[:], pr[0][:], pr[1][:], ALU.add)
                for p in pr[2:]:
                    nc.vector.tensor_tensor(acc[:], acc[:], p[:], ALU.add)
                combined[key] = acc
            return f

        def mk_reduce_mm(key):
            # matmul out must fit one PSUM bank (512 fp32) -> two 512 halves
            def f():
                rp = mmp.tile([1, L], dt.float32, tag="mm", name=f"rp_{key}")
                cp = combined[key]
                for h in range(L // 512):
                    nc.tensor.matmul(rp[:, h * 512:(h + 1) * 512], ones[:],
                                     cp[:, h * 512:(h + 1) * 512],
                                     start=True, stop=True)
                rps[key] = rp
            return f

        def mk_row(key, out_dram):
            def f():
                row = rows.tile([1, L], dt.float32, tag=f"row_{key}")
                nc.vector.tensor_copy(row[:], rps[key][:])
                nc.sync.dma_start(out=out_dram[:].rearrange("(one n) -> one n", one=1),
                                  in_=row[:])
            return f

        def zero_out(out_dram, tag):
            zr = rows.tile([1, L], dt.float32, tag=f"zr_{tag}")
            nc.vector.memset(zr[:], 0.0)
            nc.sync.dma_start(out=out_dram[:].rearrange("(one n) -> one n", one=1),
                              in_=zr[:])

        red_list = [("main", pos_main)]
        if r_self:
            red_list.append(("self", pos_self))
        else:
            zero_out(pos_self, "self")
        if r_tgt:
            red_list.append(("tgt", pos_tgt))
        else:
            zero_out(pos_tgt, "tgt")
        for key, _ in red_list:
            dq.append(mk_combine(key))

        # ---- schedule ---------------------------------------------------
        for w in range(2):
            wps = mmp.tile([128, 512], dt.float32, tag="mm", name=f"wps_{w}")
            for _ in range(4):
                nc.tensor.matmul(wps[:], wst[:], wmv[:], start=True, stop=True)

        # prologue: f1 + e2 chunks 0,1; relu consumers on ACT (same table set)
        fst_f1 = stage_chunk(f1t, 0, "f1")
        fst_c0 = stage_chunk(f2t, 0, "c0")
        fst_c1 = stage_chunk(f2t, L, "c1")
        for d in range(NDT):
            pj_cons_act(pj_mm(fst_f1, d, "f1"), e1q, d, 0)
        for d in range(NDT):
            pj_cons_act(pj_mm(fst_c0, d, "c0"), e2q, d, 0)
        for d in range(NDT):
            pj_cons_act(pj_mm(fst_c1, d, "c1"), e2q, d, L)

        # per-block DVE drain quotas (ops per projection-consumer point)
        quota = {0: 1, 1: 4, 2: 3}

        for bk in range(NCG):
            mm_slots = {}
            dv_slots = {}
            if bk < 3:
                c0, c1 = 2 * bk + 2, 2 * bk + 3
                # prefetch both chunks' staging DMAs at block start
                fsts = {c0: stage_chunk(f2t, c0 * L, f"c{c0}"),
                        c1: stage_chunk(f2t, c1 * L, f"c{c1}")}
                for idx, (cc, d) in enumerate([(c0, 0), (c0, 1), (c1, 0), (c1, 1)]):
                    m_at = (2, 4, 6, 7)[idx]

                    def mk(cc=cc, d=d, bk=bk):
                        def f():
                            ps = pj_mm(fsts[cc], d, f"c{cc}")
                            pj_cons_dve(ps, e2q, d, cc * L)
                            drain(quota[bk])
                        return f
                    mm_slots[m_at] = mk()
            else:
                # reduce-MMs right at block-3 start, rows two groups later:
                # everything evacuated by ~m5 so nothing lands in the tail
                for idx, (key, out_dram) in enumerate(red_list):
                    mm_slots[idx] = mk_reduce_mm(key)
                    dv_slots[idx + 2] = mk_row(key, out_dram)
            for m in range(8):
                sim_group(bk, m)
                if m in mm_slots:
                    mm_slots[m]()
                if m in dv_slots:
                    dv_slots[m]()
            if bk == 2:
                drain(len(dq))   # leftover banded work
                # blocks 0-2 stot columns ship now; the thin-line DMA cost
                # (128 descriptors) hides under block 3
                nc.sync.dma_start(out=s_out_a[:], in_=stot[:, 0:24])

        nc.sync.dma_start(out=s_out_b[:], in_=stot[:, 24:32])

    nc.compile()
    return nc


def kernel(feature1, feature2, W, b, positive_range_self, positive_range_tgt):
    r_self = int(np.asarray(positive_range_self))
    r_tgt = int(np.asarray(positive_range_tgt))
    assert 0 <= r_self <= PAD and 0 <= r_tgt <= PAD

    key = (r_self, r_tgt)
    if key not in _module_cache:
        _module_cache[key] = _build(r_self, r_tgt)
    nc = _module_cache[key]

    in_maps = _make_in_maps(feature1, feature2, W, b)
    res = bass_utils.run_bass_kernel_spmd(nc, in_maps, list(range(N_CORES)))

    # ---- host combine (fp64) ---------------------------------------------
    j = np.arange(L)
    loss_terms = []
    for i in range(N_CORES):
        r = res.results[i]
        Sa = r["s_out_a"].astype(np.float64).reshape(128, 3, 8).sum(axis=1)
        S = (Sa + r["s_out_b"].astype(np.float64)).T.reshape(L)
        t = SHIFT + np.log(S) - np.log(float(N))             # negative_j
        t -= r["pos_main"].astype(np.float64) / EXPA
        if r_self > 0:
            cnt = np.minimum(L - 1, j + r_self) - np.maximum(0, j - r_self) + 1.0
            t -= r["pos_self"].astype(np.float64) / EXPA / cnt
        if r_tgt > 0:
            cnt = np.minimum(L - 1, j + r_tgt) - np.maximum(0, j - r_tgt) + 1.0
            t -= r["pos_tgt"].astype(np.float64) / EXPA / cnt
        loss_terms.append(t)
    loss = np.mean(np.concatenate(loss_terms))
    return np.float32(loss)


def _make_in_maps(feature1, feature2, W, b):
    fp8 = ml_dtypes.float8_e4m3
    f1 = np.asarray(feature1, dtype=np.float32)
    f2 = np.asarray(feature2, dtype=np.float32)
    Wr = np.asarray(W, dtype=np.float32) * SQA
    bv0 = np.asarray(b, dtype=np.float32) * SQA
    bv = np.ascontiguousarray(bv0.reshape(NDT, 128).T)      # [128, NDT]

    # W: [DIN, DF] -> [128, KO*DF] partition-major (w[p, ko*DF+d] = W[ko*128+p, d])
    w_pm = np.ascontiguousarray(
        Wr.reshape(KO, 128, DF).transpose(1, 0, 2).reshape(128, KO * DF).astype(fp8))

    # f2: [B, L, DIN] -> [128, c, KO, L] chunk-contiguous partition-major
    f2T = f2.reshape(N, DIN).T.astype(fp8)                   # [DIN, N]
    f2_4d = (f2T.reshape(KO, 128, B, L).transpose(1, 2, 0, 3))   # [p, c, ko, n]
    in_maps = []
    for i in range(N_CORES):
        f1T = f1[i].T.astype(fp8)                            # [DIN, L]
        f1_pm = np.ascontiguousarray(
            f1T.reshape(KO, 128, L).transpose(1, 0, 2).reshape(128, KO * L))
        f2_rot = np.ascontiguousarray(
            np.roll(f2_4d, -i, axis=1).reshape(128, B * KO * L))
        in_maps.append({"f1t": f1_pm, "f2t": f2_rot, "w_in": w_pm, "b_in": bv})
    return in_maps
